# revision 13
# baseline (speedup 1.0000x reference)
"""CrossFusion block on 8 TRN2 NeuronCores.

Data-parallel over batch: 64 batches -> 8 cores x 8 batches.
All activations kept feature-major ("X.T" layout, feature dim on SBUF
partitions) so every matmul chains without any on-device transposes; all
layout transposition is done host-side while sharding.

The projection matmuls q/k/v/o/f1/f2 run in fp8(e4m3) DoubleRow perf
mode (two k-rows per PE pass = 2x bf16 throughput). Weights are scaled
x32 host-side into fp8's normal range; the descale is folded into the
PSUM-readout activation (out = func(psum/32 + bias)). The wide llm
projection (K=2048) and the attention itself stay bf16: fp8 there costs
too much accuracy (proj feeds the residual stream directly).

LayerNorm affine (g, b) is folded host-side into the consuming weight
matrices (W <- W*diag(g), bias <- bias + W@b), so the device only
computes the pure normalization z = x*A + B with A=1/std, B=-mean/std
broadcast from rank-1 matmuls; the apply is 2 DVE ops and the fp8
quantization of the normalized activations happens once, on the final
write. LayerNorm statistics are computed with ones-vector matmuls on
the TensorEngine (partition-dim reductions).

Softmax skips max-subtraction (scores are O(1) here), normalizes via a
PE rank-1 broadcast of 16/rowsum; the extra x16 puts the attention
output into fp8's sweet spot for the o-projection input.
"""

import sys

sys.path.insert(0, "/opt/trn_rl_repo")

import numpy as np
import ml_dtypes

import concourse.bass as bass
import concourse.tile as tile
from concourse import bacc, mybir
from concourse import bass_utils

BF16 = ml_dtypes.bfloat16
NPF8 = ml_dtypes.float8_e4m3

B, LC, LL, LLAMA_DIM, DIM, HEADS = 64, 77, 256, 2048, 768, 8
HEAD_DIM = DIM // HEADS          # 96
SCALE = HEAD_DIM ** -0.5
FF = 4 * DIM                     # 3072
NCORES = 8
BPC = B // NCORES                # batches per core = 8
TQ = BPC * LL                    # llm tokens per core = 2048
TK = BPC * LC                    # clip tokens per core = 616
KT_D = DIM // 128                # 6
KT_L = LLAMA_DIM // 128          # 16
KT_F = FF // 128                 # 24
EPS = 1e-5

F32 = mybir.dt.float32
BF = mybir.dt.bfloat16
F8 = mybir.dt.float8e4
PM = mybir.MatmulPerfMode.DoubleRow
AF = mybir.ActivationFunctionType
OP = mybir.AluOpType

FP8_QKVO = True                  # q/k/v/o matmuls in fp8 DoubleRow
FP8_F1 = True                    # FFN up-proj in fp8 DoubleRow
FP8_F2 = True                    # FFN down-proj in fp8 DoubleRow
WS = 32.0                        # fp8 weight pre-scale
OS = 16.0                        # softmax 16/rowsum scale (fp8 range aid)

# packed param tile column offsets (all f32, [128, PP_COLS])
PP_PROJB, PP_OB, PP_F2B = 0, 6, 12
PP_F1B = 18          # 24 cols
PP_QB = 42           # 8 cols (rows 0..95)
PP_KB = 50           # 8 cols (rows 0..95)
PP_EPS = 58
PP_COLS = 60


def _dt(flag):
    return F8 if flag else BF


def _np_dt(flag):
    return NPF8 if flag else BF16


def _ws(flag):
    return WS if flag else 1.0


def _ln_stats_chunk(nc, pools, x_sb, n_kt, c0, cw, a_row, b_row, ptmp, psum):
    """Emit LN stats for columns [c0, c0+cw) of feature-major x_sb.

    a_row gets 1/std, b_row gets -mean/std for each column."""
    ones128 = pools["ones128"]
    ps_s = psum.tile([128, 512], F32, tag="ps", name=f"ps_s{c0}")
    ps_q = psum.tile([128, 512], F32, tag="ps", name=f"ps_q{c0}")
    for kt in range(n_kt):
        xs = x_sb[:, kt, c0:c0 + cw]
        sq = ptmp.tile([128, 512], BF, tag="sq")
        # square on DVE (all-SBUF bf16 -> high-rate) to keep ACT free
        nc.vector.tensor_tensor(out=sq[:, :cw], in0=xs, in1=xs, op=OP.mult)
        nc.tensor.matmul(ps_s[:1, :cw], ones128, xs,
                         start=(kt == 0), stop=(kt == n_kt - 1))
        nc.tensor.matmul(ps_q[:1, :cw], ones128, sq[:, :cw],
                         start=(kt == 0), stop=(kt == n_kt - 1))
    # rs slices: [0:512) m, [512:1024) mq, [1024:1536) var/std/inv
    rs = ptmp.tile([1, 1536], F32, tag="rs")
    m = rs[:, 0:cw]
    mq = rs[:, 512:512 + cw]
    c = rs[:, 1024:1024 + cw]
    nc.scalar.activation(out=m, in_=ps_s[:1, :cw], func=AF.Copy,
                         scale=1.0 / DIM)
    nc.scalar.activation(out=mq, in_=ps_q[:1, :cw], func=AF.Copy,
                         scale=1.0 / DIM)
    nc.vector.tensor_tensor(out=c, in0=m, in1=m, op=OP.mult)
    nc.vector.tensor_tensor(out=c, in0=mq, in1=c, op=OP.subtract)
    nc.scalar.activation(out=c, in_=c, func=AF.Sqrt, bias=pools["eps1"])
    nc.vector.reciprocal(out=c, in_=c)
    nc.vector.tensor_copy(out=a_row[:, c0:c0 + cw], in_=c)
    nc.vector.tensor_scalar(out=m, in0=m, scalar1=-1.0, scalar2=None,
                            op0=OP.mult)
    nc.vector.tensor_tensor(out=b_row[:, c0:c0 + cw], in0=m, in1=c,
                            op=OP.mult)


def _ln_norm_chunk(nc, pools, x_view, out_view, n_kt, cw, a_row, b_row, c0,
                   ptmp, psum):
    """Normalize one column chunk: out = x*A + B (feature-major).

    Writes out_view (any dtype, e.g. fp8) exactly once per element; the
    multiply intermediate goes through a bf16 scratch."""
    ones1 = pools["ones1"]
    uniq = f"{c0}_{x_view.tensor.name}"
    ps_a = psum.tile([128, 512], F32, tag="ps", name=f"ps_a_{uniq}")
    ps_b = psum.tile([128, 512], F32, tag="ps", name=f"ps_b_{uniq}")
    nc.tensor.matmul(ps_a[:, :cw], ones1, a_row[:, c0:c0 + cw])
    nc.tensor.matmul(ps_b[:, :cw], ones1, b_row[:, c0:c0 + cw])
    # drain the broadcasts to SBUF bf16 once (split across ACT + DVE) so
    # the per-k-tile apply runs on DVE in its all-SBUF 16-bit fast mode
    ab = ptmp.tile([128, 1024], BF, tag="ab")
    nc.scalar.activation(out=ab[:, 0:cw], in_=ps_a[:, :cw], func=AF.Copy)
    nc.vector.tensor_copy(out=ab[:, 512:512 + cw], in_=ps_b[:, :cw])
    for kt in range(n_kt):
        tmp = ptmp.tile([128, 512], BF, tag="lnt")
        nc.vector.tensor_tensor(out=tmp[:, :cw], in0=x_view[:, kt, :cw],
                                in1=ab[:, 0:cw], op=OP.mult)
        nc.vector.tensor_tensor(out=out_view[:, kt, :cw], in0=tmp[:, :cw],
                                in1=ab[:, 512:512 + cw], op=OP.add)


def _mm_acc(nc, ps, w_sb, x_sb, n_kt, mt_slice, c_slice, fp8):
    """Accumulate w[:, :, mt_slice].T @ x[:, :, c_slice] over k-tiles
    into psum ps; DoubleRow pairs when fp8."""
    if fp8:
        for j in range(n_kt // 2):
            nc.tensor.matmul(
                ps, w_sb[:, 2 * j:2 * j + 2, mt_slice],
                x_sb[:, 2 * j:2 * j + 2, c_slice],
                start=(j == 0), stop=(j == n_kt // 2 - 1), perf_mode=PM)
    else:
        for kt in range(n_kt):
            nc.tensor.matmul(
                ps, w_sb[:, kt, mt_slice], x_sb[:, kt, c_slice],
                start=(kt == 0), stop=(kt == n_kt - 1))


def build_nc():
    nc = bacc.Bacc("TRN2", target_bir_lowering=False, debug=False)

    dt_qkvo = _dt(FP8_QKVO)
    dt_f1 = _dt(FP8_F1)
    dt_f2 = _dt(FP8_F2)

    embT = nc.dram_tensor("embT", (KT_L, 128, TQ), BF, kind="ExternalInput")
    clipT = nc.dram_tensor("clipT", (KT_D, 128, TK), BF, kind="ExternalInput")
    wprojT = nc.dram_tensor("wprojT", (KT_L, 128, DIM), BF, kind="ExternalInput")
    wqT = nc.dram_tensor("wqT", (KT_D, 128, DIM), dt_qkvo, kind="ExternalInput")
    wkT = nc.dram_tensor("wkT", (KT_D, 128, DIM), dt_qkvo, kind="ExternalInput")
    wvT = nc.dram_tensor("wvT", (KT_D, 128, DIM), dt_qkvo, kind="ExternalInput")
    woT = nc.dram_tensor("woT", (HEAD_DIM, HEADS, DIM), dt_qkvo,
                         kind="ExternalInput")
    f1T = nc.dram_tensor("f1T", (KT_D, 128, FF), dt_f1, kind="ExternalInput")
    f2T = nc.dram_tensor("f2T", (KT_F, 128, DIM), dt_f2, kind="ExternalInput")
    pp = nc.dram_tensor("pp", (128, PP_COLS), F32, kind="ExternalInput")
    outT = nc.dram_tensor("outT", (KT_D, 128, TQ), F32, kind="ExternalOutput")

    iws_qkvo = 1.0 / _ws(FP8_QKVO)
    iws_f1 = 1.0 / _ws(FP8_F1)
    iws_f2 = 1.0 / _ws(FP8_F2)
    NCH = 512
    NFC = TQ // NCH

    with tile.TileContext(nc) as tc:
        from contextlib import ExitStack
        with ExitStack() as stk:
            pw = stk.enter_context(tc.tile_pool(name="pw", bufs=1))
            pact = stk.enter_context(tc.tile_pool(name="pact", bufs=1))
            prow = stk.enter_context(tc.tile_pool(name="prow", bufs=1))
            ptmp = stk.enter_context(tc.tile_pool(name="ptmp", bufs=2))
            psum = stk.enter_context(
                tc.tile_pool(name="psum", bufs=8, space="PSUM"))
            pqw = stk.enter_context(tc.tile_pool(name="pqw", bufs=1))
            pmid = stk.enter_context(tc.tile_pool(name="pmid", bufs=2))
            patn = stk.enter_context(tc.tile_pool(name="patn", bufs=2))
            pffn = stk.enter_context(tc.tile_pool(name="pffn", bufs=2))
            pfc = stk.enter_context(tc.tile_pool(name="pfc", bufs=1))
            pout = stk.enter_context(tc.tile_pool(name="pout", bufs=2))

            ones_sq = pw.tile([128, 128], BF, tag="ones")
            nc.vector.memset(ones_sq, 1.0)
            ones128 = ones_sq[:, 0:1]
            ones1 = ones_sq[0:1, :]
            s16_row = pw.tile([1, 128], BF, tag="s16")
            nc.vector.memset(s16_row, OS)
            pp_sb = pw.tile([128, PP_COLS], F32, tag="pp")
            nc.sync.dma_start(out=pp_sb, in_=pp.ap())
            pools = {"ones128": ones128, "ones1": ones1,
                     "eps1": pp_sb[:1, PP_EPS:PP_EPS + 1]}

            def ppc(col, n=1, rows=128):
                return pp_sb[:rows, col:col + n]

            def load3(pool, dram, shape, name):
                t = pool.tile(list(shape), dram.dtype, tag=name)
                for k in range(shape[1]):
                    nc.sync.dma_start(out=t[:, k, :], in_=dram.ap()[k])
                return t

            llm_sb = pact.tile([128, KT_D, TQ], BF, tag="llm")     # llm.T/llm2.T
            k_sb = pact.tile([HEAD_DIM, HEADS, TK], BF, tag="k")   # k.T hd-major
            v_sb = pact.tile([LC, BPC, DIM], BF, tag="v")          # v tok-major

            a_kv = prow.tile([1, TQ], BF, tag="a_row", name="a_kv")
            b_kv = prow.tile([1, TQ], BF, tag="b_row", name="b_kv")
            a_2 = prow.tile([1, TQ], BF, tag="a2_row", name="a_2")
            b_2 = prow.tile([1, TQ], BF, tag="b2_row", name="b_2")

            # The whole block runs as one software pipeline: PE-dense work
            # (last proj chunk, FFN chunks) is put on a filler queue and
            # emitted inside the latency-bound attention softmax chains, so
            # the in-order PE queue always has dependency-ready matmuls to
            # chew on while softmax round-trips through ACT/DVE.
            filler = []

            def fill(n=1):
                for _ in range(min(n, len(filler))):
                    filler.pop(0)()

            # ====== clip LN + k + v (small, fills engines during DMA) ======
            with tc.tile_pool(name="pkvw", bufs=1) as pkvw, \
                 tc.tile_pool(name="pclip", bufs=1) as pclip:
                clip_sb = load3(pclip, clipT, (128, KT_D, TK), "clip")
                wk_sb = load3(pkvw, wkT, (128, KT_D, DIM), "wk")
                wv_sb = load3(pkvw, wvT, (128, KT_D, DIM), "wv")
                clipn_sb = pclip.tile([128, KT_D, TK], dt_qkvo, tag="clipn")

                a_c = prow.tile([1, TK], BF, tag="ac_row", name="a_c")
                b_c = prow.tile([1, TK], BF, tag="bc_row", name="b_c")
                for ci in range(2):
                    _ln_stats_chunk(nc, pools, clip_sb, KT_D, ci * 308, 308,
                                    a_c, b_c, ptmp, psum)
                for ci in range(2):
                    c0 = ci * 308
                    _ln_norm_chunk(nc, pools, clip_sb[:, :, c0:c0 + 308],
                                   clipn_sb[:, :, c0:c0 + 308], KT_D, 308,
                                   a_c, b_c, c0, ptmp, psum)

                # k.T head-major [96, h, 616]
                for h in range(HEADS):
                    for ci in range(2):
                        c0 = ci * 308
                        ps = psum.tile([128, 512], F32, tag="ps")
                        _mm_acc(nc, ps[:HEAD_DIM, :308], wk_sb, clipn_sb,
                                KT_D, slice(h * 96, (h + 1) * 96),
                                slice(c0, c0 + 308), FP8_QKVO)
                        nc.scalar.activation(
                            out=k_sb[:, h, c0:c0 + 308],
                            in_=ps[:HEAD_DIM, :308],
                            func=AF.Identity, scale=iws_qkvo,
                            bias=ppc(PP_KB + h, rows=96))

                # v token-major [77, b, 768]; the v bias rides through the
                # softmax average (attn weights sum to 1) and is folded into
                # the o-projection bias host-side. The 77-col stationary
                # operand cannot DoubleRow (odd cols), so plain fp8 matmuls.
                for b in range(BPC):
                    for ci in range(2):
                        c0 = ci * 384
                        ps = psum.tile([128, 512], F32, tag="ps")
                        for kt in range(KT_D):
                            nc.tensor.matmul(
                                ps[:LC, :384],
                                clipn_sb[:, kt, b * LC:(b + 1) * LC],
                                wv_sb[:, kt, c0:c0 + 384],
                                start=(kt == 0), stop=(kt == KT_D - 1))
                        nc.scalar.activation(out=v_sb[:, b, c0:c0 + 384],
                                             in_=ps[:LC, :384],
                                             func=AF.Identity, scale=iws_qkvo)

            wq_sb = load3(pqw, wqT, (128, KT_D, DIM), "wq")
            wo_sb = pqw.tile([HEAD_DIM, HEADS, DIM], dt_qkvo, tag="wo")
            for h in range(HEADS):
                nc.sync.dma_start(out=wo_sb[:, h, :], in_=woT.ap()[:, h, :])
            ffn_w = {}

            # ==================== per-batch emitters =======================
            lnns = {}

            def emit_lnn(bb):
                t = pmid.tile([128, KT_D, LL], dt_qkvo, tag="lnn",
                              name=f"lnn{bb}")
                cc = bb * LL
                _ln_norm_chunk(nc, pools, llm_sb[:, :, cc:cc + LL],
                               t, KT_D, LL, a_kv, b_kv, cc, ptmp, psum)
                lnns[bb] = t

            def emit_q(bb):
                lnn = lnns.pop(bb)
                t = pmid.tile([HEAD_DIM, HEADS * LL], BF, tag="q_c",
                              name=f"q_c{bb}")
                for pr in range(HEADS // 2):
                    ps = psum.tile([128, 512], F32, tag="ps",
                                   name=f"ps_qp_{bb}_{pr}")
                    for i in range(2):
                        h = 2 * pr + i
                        _mm_acc(nc, ps[:HEAD_DIM, i * LL:(i + 1) * LL],
                                wq_sb, lnn, KT_D,
                                slice(h * 96, (h + 1) * 96), slice(0, LL),
                                FP8_QKVO)
                    fill()
                    for i in range(2):
                        h = 2 * pr + i
                        nc.scalar.activation(
                            out=t[:, h * LL:(h + 1) * LL],
                            in_=ps[:HEAD_DIM, i * LL:(i + 1) * LL],
                            func=AF.Identity, scale=iws_qkvo,
                            bias=ppc(PP_QB + h, rows=96))
                return t

            def emit_attn(b, q_c, ao_c):
                # pairs of heads; per pair 3 psum banks:
                #   ps1: scores h0|h1 [0:77, 0:256|256:512], then av out
                #        [0:96, 0:256|256:512] (regions reused after exp)
                #   pss: softmax sums [0:1, 0:512]
                #   psb: 16/sum bcast [0:77, 0:512]
                ex_g = patn.tile([LC, HEADS * LL], BF, tag="exg",
                                 name=f"exg_{b}")
                for g in range(HEADS // 2):
                    h0, h1 = 2 * g, 2 * g + 1
                    ps1 = psum.tile([128, 512], F32, tag="ps",
                                    name=f"ps1_{b}_{g}")
                    for i, h in enumerate((h0, h1)):
                        nc.tensor.matmul(ps1[:LC, i * LL:(i + 1) * LL],
                                         k_sb[:, h, b * LC:(b + 1) * LC],
                                         q_c[:, h * LL:(h + 1) * LL])
                    fill()
                    for i, h in enumerate((h0, h1)):
                        nc.scalar.activation(
                            out=ex_g[:, h * LL:(h + 1) * LL],
                            in_=ps1[:LC, i * LL:(i + 1) * LL],
                            func=AF.Exp, scale=SCALE)
                    pss = psum.tile([128, 512], F32, tag="ps",
                                    name=f"pss_{b}_{g}")
                    for i, h in enumerate((h0, h1)):
                        nc.tensor.matmul(pss[:1, i * LL:(i + 1) * LL],
                                         ones128[:LC, :],
                                         ex_g[:, h * LL:(h + 1) * LL])
                    fill()
                    inv = patn.tile([1, 2 * LL], BF, tag="inv",
                                    name=f"inv_{b}_{g}")
                    with nc.allow_low_precision("softmax 1/sum bf16"):
                        nc.vector.reciprocal(out=inv, in_=pss[:1, :2 * LL])
                    psb = psum.tile([128, 512], F32, tag="ps",
                                    name=f"psb_{b}_{g}")
                    nc.tensor.matmul(psb[:LC, :2 * LL], s16_row[:, :LC], inv)
                    fill()
                    nc.vector.tensor_tensor(
                        out=ex_g[:, h0 * LL:(h0 + 2) * LL],
                        in0=ex_g[:, h0 * LL:(h0 + 2) * LL],
                        in1=psb[:LC, :2 * LL], op=OP.mult)
                    for i, h in enumerate((h0, h1)):
                        nc.tensor.matmul(
                            ps1[:HEAD_DIM, i * LL:(i + 1) * LL],
                            v_sb[:, b, h * 96:(h + 1) * 96],
                            ex_g[:, h * LL:(h + 1) * LL])
                    fill()
                    for i, h in enumerate((h0, h1)):
                        nc.scalar.activation(
                            out=ao_c[:, h, :],
                            in_=ps1[:HEAD_DIM, i * LL:(i + 1) * LL],
                            func=AF.Identity)

            def emit_o(b, ao_c):
                # o-proj + bias + residual (in-place: llm becomes llm2).
                # bias is pre-added to llm via gpsimd; the psum readout then
                # fuses the 1/(WS*OS) descale with the residual add.
                c0 = b * LL
                for mt in range(KT_D):
                    ps = psum.tile([128, 512], F32, tag="ps")
                    if FP8_QKVO:
                        for i in range(HEADS // 2):
                            nc.tensor.matmul(
                                ps[:, :LL],
                                wo_sb[:, 2 * i:2 * i + 2,
                                      mt * 128:(mt + 1) * 128],
                                ao_c[:, 2 * i:2 * i + 2, :],
                                start=(i == 0), stop=(i == HEADS // 2 - 1),
                                perf_mode=PM)
                    else:
                        for h in range(HEADS):
                            nc.tensor.matmul(
                                ps[:, :LL],
                                wo_sb[:, h, mt * 128:(mt + 1) * 128],
                                ao_c[:, h, :],
                                start=(h == 0), stop=(h == HEADS - 1))
                    fill()
                    nc.gpsimd.tensor_scalar(
                        out=llm_sb[:, mt, c0:c0 + LL],
                        in0=llm_sb[:, mt, c0:c0 + LL],
                        scalar1=ppc(PP_OB + mt), scalar2=None, op0=OP.add)
                    nc.vector.scalar_tensor_tensor(
                        out=llm_sb[:, mt, c0:c0 + LL],
                        in0=ps[:, :LL],
                        scalar=iws_qkvo / OS,
                        in1=llm_sb[:, mt, c0:c0 + LL],
                        op0=OP.mult, op1=OP.add)

            def batch_block(b, tail=True):
                emit_lnn(b)
                fill()
                q_c = emit_q(b)
                ao_c = pmid.tile([HEAD_DIM, HEADS, LL], dt_qkvo, tag="ao_c",
                                 name=f"ao_c{b}")
                emit_attn(b, q_c, ao_c)
                emit_o(b, ao_c)
                if tail and b % 2 == 1:
                    ci = b // 2
                    _ln_stats_chunk(nc, pools, llm_sb, KT_D,
                                    ci * NCH, NCH, a_2, b_2, ptmp, psum)
                    enqueue_ffn(ci)

            # ======================= FFN packets ===========================
            def enqueue_ffn(ci):
                c0 = ci * NCH
                h_c = pffn.tile([128, KT_D, NCH], dt_f1, tag="h_c",
                                name=f"h_c{ci}")
                _ln_norm_chunk(nc, pools, llm_sb[:, :, c0:c0 + NCH],
                               h_c, KT_D, NCH, a_2, b_2, c0, ptmp, psum)
                # pre-add the f2 bias into the residual on gpsimd (the LN2
                # stats and h for this chunk are already computed)
                for mt in range(KT_D):
                    nc.gpsimd.tensor_scalar(
                        out=llm_sb[:, mt, c0:c0 + NCH],
                        in0=llm_sb[:, mt, c0:c0 + NCH],
                        scalar1=ppc(PP_F2B + mt), scalar2=None, op0=OP.add)
                f_c = pfc.tile([128, KT_F, NCH], dt_f2, tag="f_c",
                               name=f"f_c{ci}")

                def f1_packet(mt, h_c=h_c, f_c=f_c):
                    ps = psum.tile([128, 512], F32, tag="ps")
                    _mm_acc(nc, ps[:, :NCH], ffn_w["f1"], h_c, KT_D,
                            slice(mt * 128, (mt + 1) * 128),
                            slice(0, NCH), FP8_F1)
                    nc.scalar.activation(
                        out=f_c[:, mt, :], in_=ps[:, :NCH],
                        func=AF.Gelu_apprx_sigmoid, scale=iws_f1,
                        bias=ppc(PP_F1B + mt))

                def f2_packet(mt, f_c=f_c, c0=c0):
                    ps = psum.tile([128, 512], F32, tag="ps")
                    _mm_acc(nc, ps[:, :NCH], ffn_w["f2"], f_c, KT_F,
                            slice(mt * 128, (mt + 1) * 128),
                            slice(0, NCH), FP8_F2)
                    o_c = pout.tile([128, NCH], F32, tag="o_c")
                    nc.vector.scalar_tensor_tensor(
                        out=o_c, in0=ps[:, :NCH], scalar=iws_f2,
                        in1=llm_sb[:, mt, c0:c0 + NCH],
                        op0=OP.mult, op1=OP.add)
                    nc.sync.dma_start(out=outT.ap()[mt, :, c0:c0 + NCH],
                                      in_=o_c)

                for mt in range(KT_F):
                    filler.append(lambda mt=mt: f1_packet(mt))
                for mt in range(KT_D):
                    filler.append(lambda mt=mt: f2_packet(mt))

            # ================= proj + pipelined main loop ==================
            NCP = 256
            with tc.tile_pool(name="pwproj", bufs=1) as pwproj, \
                 tc.tile_pool(name="pemb", bufs=2) as pemb:
                wp_sb = load3(pwproj, wprojT, (128, KT_L, DIM), "wproj")

                def proj_chunk_packets(ci):
                    c0 = ci * NCP
                    emb_c = pemb.tile([128, KT_L, NCP], BF, tag="emb_c")
                    for kt in range(KT_L):
                        nc.sync.dma_start(out=emb_c[:, kt, :],
                                          in_=embT.ap()[kt, :, c0:c0 + NCP])
                    pk = []
                    for mt in range(KT_D):
                        def p(mt=mt, emb_c=emb_c, c0=c0):
                            ps = psum.tile([128, 512], F32, tag="ps")
                            _mm_acc(nc, ps[:, :NCP], wp_sb, emb_c, KT_L,
                                    slice(mt * 128, (mt + 1) * 128),
                                    slice(0, NCP), False)
                            nc.scalar.activation(
                                out=llm_sb[:, mt, c0:c0 + NCP],
                                in_=ps[:, :NCP],
                                func=AF.Identity, bias=ppc(PP_PROJB + mt))
                        pk.append(p)
                    pk.append(lambda c0=c0: _ln_stats_chunk(
                        nc, pools, llm_sb, KT_D, c0, NCP, a_kv, b_kv,
                        ptmp, psum))
                    return pk

                for ci in range(TQ // NCP - 1):
                    for p in proj_chunk_packets(ci):
                        p()
                # last proj chunk becomes filler inside batch 0
                filler.extend(proj_chunk_packets(TQ // NCP - 1))

                batch_block(0)
                batch_block(1, tail=False)
                fill(len(filler))  # drain any leftover proj packets

            pfw = stk.enter_context(tc.tile_pool(name="pfw", bufs=1))
            ffn_w["f1"] = load3(pfw, f1T, (128, KT_D, FF), "f1")
            ffn_w["f2"] = load3(pfw, f2T, (128, KT_F, DIM), "f2")

            # chunk-0 LN2 stats + FFN enqueue (outside the proj pools so the
            # FFN packets stay queued as filler for batches 2-3)
            _ln_stats_chunk(nc, pools, llm_sb, KT_D, 0, NCH, a_2, b_2,
                            ptmp, psum)
            enqueue_ffn(0)
            for b in range(2, BPC):
                batch_block(b)
            fill(len(filler))      # drain the tail FFN chunks

    nc.compile()
    return nc


_NC_CACHE = {}


def _get_nc():
    if "nc" not in _NC_CACHE:
        _NC_CACHE["nc"] = build_nc()
    return _NC_CACHE["nc"]


def _prep_in_maps(inputs):
    f32 = np.float32

    def bf(x):
        return np.ascontiguousarray(x).astype(BF16)

    def q8(x, s):
        return np.ascontiguousarray(np.asarray(x, f32) * s).astype(NPF8)

    def wcast(x, flag):
        if flag:
            return np.ascontiguousarray(np.asarray(x, f32) * WS).astype(NPF8)
        return bf(x)

    # Fold LN affine params into the consuming projections (exact in f32):
    #   y = (z*g + b) @ W.T + c  ==  z @ (W*g).T + (c + W @ b)
    qn_g = np.asarray(inputs["qn_g"], f32)
    qn_b = np.asarray(inputs["qn_b"], f32)
    kvn_g = np.asarray(inputs["kvn_g"], f32)
    kvn_b = np.asarray(inputs["kvn_b"], f32)
    n_g = np.asarray(inputs["n_g"], f32)
    n_b = np.asarray(inputs["n_b"], f32)

    q_w = np.asarray(inputs["q_w"], f32) * kvn_g[None, :]
    q_b = np.asarray(inputs["q_b"], f32) + np.asarray(inputs["q_w"], f32) @ kvn_b
    k_w = np.asarray(inputs["k_w"], f32) * qn_g[None, :]
    k_b = np.asarray(inputs["k_b"], f32) + np.asarray(inputs["k_w"], f32) @ qn_b
    v_w = np.asarray(inputs["v_w"], f32) * qn_g[None, :]
    v_b = np.asarray(inputs["v_b"], f32) + np.asarray(inputs["v_w"], f32) @ qn_b
    # v bias rides through the softmax average (weights sum to 1): fold it
    # into the o-projection bias instead of adding it to every v row.
    o_b = np.asarray(inputs["o_b"], f32) + np.asarray(inputs["o_w"], f32) @ v_b
    f1_w = np.asarray(inputs["f1_w"], f32) * n_g[None, :]
    f1_b = np.asarray(inputs["f1_b"], f32) + np.asarray(inputs["f1_w"], f32) @ n_b

    w = {}
    w["wprojT"] = bf(inputs["llm_proj_w"].astype(f32).T.reshape(KT_L, 128, DIM))
    w["wqT"] = wcast(q_w.T.reshape(KT_D, 128, DIM), FP8_QKVO)
    w["wkT"] = wcast(k_w.T.reshape(KT_D, 128, DIM), FP8_QKVO)
    w["wvT"] = wcast(v_w.T.reshape(KT_D, 128, DIM), FP8_QKVO)
    w["woT"] = wcast(np.ascontiguousarray(
        inputs["o_w"].astype(f32).T.reshape(HEADS, HEAD_DIM, DIM)
        .transpose(1, 0, 2)), FP8_QKVO)
    w["f1T"] = wcast(f1_w.T.reshape(KT_D, 128, FF), FP8_F1)
    w["f2T"] = wcast(inputs["f2_w"].astype(f32).T.reshape(KT_F, 128, DIM),
                     FP8_F2)

    ppa = np.zeros((128, PP_COLS), dtype=f32)

    def put(col, vec, n):
        ppa[:, col:col + n] = np.asarray(vec, dtype=f32).reshape(n, 128).T

    put(PP_PROJB, inputs["llm_proj_b"], KT_D)
    put(PP_OB, o_b, KT_D)
    put(PP_F2B, inputs["f2_b"], KT_D)
    put(PP_F1B, f1_b, KT_F)
    ppa[:HEAD_DIM, PP_QB:PP_QB + HEADS] = q_b.reshape(HEADS, HEAD_DIM).T
    ppa[:HEAD_DIM, PP_KB:PP_KB + HEADS] = k_b.reshape(HEADS, HEAD_DIM).T
    ppa[:, PP_EPS] = EPS
    w["pp"] = ppa

    clip = np.asarray(inputs["clip_embed"], dtype=f32)
    llm = np.asarray(inputs["llm_embed"], dtype=f32)
    in_maps = []
    for c in range(NCORES):
        cs = slice(c * BPC, (c + 1) * BPC)
        m = dict(w)
        m["embT"] = bf(llm[cs].reshape(TQ, LLAMA_DIM).T.reshape(KT_L, 128, TQ))
        m["clipT"] = bf(clip[cs].reshape(TK, DIM).T.reshape(KT_D, 128, TK))
        in_maps.append(m)
    return in_maps


def run(inputs, trace=False):
    nc = _get_nc()
    in_maps = _prep_in_maps(inputs)
    res = bass_utils.run_bass_kernel_spmd(
        nc, in_maps, core_ids=list(range(NCORES)), trace=trace)
    clip = np.asarray(inputs["clip_embed"], dtype=np.float32)
    llm3 = np.empty((B, LL, DIM), dtype=np.float32)
    for c in range(NCORES):
        yT = res.results[c]["outT"].reshape(DIM, TQ)
        llm3[c * BPC:(c + 1) * BPC] = yT.T.reshape(BPC, LL, DIM)
    out = np.concatenate([clip, llm3], axis=1)
    return out, res


def kernel(**inputs):
    out, _ = run(inputs, trace=False)
    return out
